# revision 5
# baseline (speedup 1.0000x reference)
"""Trainium2 Bass kernel for nn_Attention_695784702572.

Model (per batch b, head h):
    z      = layernorm(features)                      (standardize; gamma/beta folded into weights)
    q,k,v  = z @ Wq, z @ Wk, z @ Wv                   (per-head slices of w_qkv)
    S      = q @ k^T * DH^-0.5
    P      = softmax(S)   (rows sum to 1)
    agg    = P @ delta = P @ xyz - xyz                (since softmax rows sum to 1)
    out_h  = P @ v + (P @ xyz - xyz) @ w_sp
           = P @ (v + xyz @ w_sp) - xyz @ w_sp
    y      = gelu(concat_h(out_h) @ w_out + b_out) + features

Sharding: 16 (b,h) pairs over 8 cores -> core c handles batch c//4, heads
2*(c%4), 2*(c%4)+1 for the attention launch.  The final projection mixes
heads, so a second SPMD launch row-shards the 4096 token rows 512/core.
The "- xyz @ w_sp" term is identical for every head so it is folded into
the final projection as  - s_b @ (sum_h w_out[64h:64h+64])  (exact algebra).

All matmuls run as fp32r (TF32-like, full PE rate at N>=256); softmax skips
the max-subtraction (|S*scale| < ~6 for these inputs, exp is safe in fp32);
the softmax row-sum comes for free from a ones-column appended to v.
"""

import numpy as np

import concourse.bacc as bacc
import concourse.bass as bass
import concourse.tile as tile
from concourse import mybir
from concourse.bass_utils import run_bass_kernel_spmd

F32 = mybir.dt.float32
F32R = mybir.dt.float32r
EXP = mybir.ActivationFunctionType.Exp
GELU = mybir.ActivationFunctionType.Gelu
SQRT = mybir.ActivationFunctionType.Sqrt
MULT = mybir.AluOpType.mult
ADD = mybir.AluOpType.add

B, L, N, DIM = 2, 8, 256, 512
M = L * N            # 2048 tokens per batch
H, DH = 8, 64
INNER = H * DH       # 512
EPS = 1e-5
SCALE = DH ** -0.5
NCORES = 8
MT = M // 128        # 16 m-tiles
KT = DIM // 128      # 4 contraction tiles
NCH = M // 512       # 4 i-chunks


def _build_launch1() -> bass.Bass:
    """Per core: batch b, 2 heads. In: x[2048,512], xyzT[3,2048], weights.
    Out: otn[128,2048] = normalized (P@v')^T for the 2 heads (rows 0-63 head
    A, 64-127 head B), where v' = v + xyz@w_sp."""
    nc = bacc.Bacc()
    x = nc.dram_tensor("x", [M, DIM], F32, kind="ExternalInput")
    xyzT = nc.dram_tensor("xyzT", [3, M], F32R, kind="ExternalInput")
    wqk = nc.dram_tensor("wqk", [DIM, 256], F32R, kind="ExternalInput")
    wv = nc.dram_tensor("wv", [DIM, 128], F32R, kind="ExternalInput")
    wsp = nc.dram_tensor("wsp", [3, DH], F32R, kind="ExternalInput")
    bqk = nc.dram_tensor("bqk", [128, 2], F32, kind="ExternalInput")
    bv = nc.dram_tensor("bv", [128, 1], F32, kind="ExternalInput")
    idm = nc.dram_tensor("idm", [128, 128], F32R, kind="ExternalInput")
    otn = nc.dram_tensor("otn", [128, M], F32, kind="ExternalOutput")

    with tile.TileContext(nc) as tc:
        with tc.tile_pool(name="consts", bufs=1) as consts, \
             tc.tile_pool(name="xall", bufs=1) as xall, \
             tc.tile_pool(name="stats", bufs=1) as stats, \
             tc.tile_pool(name="zp", bufs=3) as zp, \
             tc.tile_pool(name="ztp", bufs=1) as ztp, \
             tc.tile_pool(name="qkvsb", bufs=1) as qkvsb, \
             tc.tile_pool(name="esb", bufs=3) as esb, \
             tc.tile_pool(name="otp", bufs=1) as otp, \
             tc.tile_pool(name="small", bufs=4) as small:

            wqk_sb = consts.tile([128, KT, 256], F32R)
            wv_sb = consts.tile([128, KT, 128], F32R)
            wsp_sb = consts.tile([3, DH], F32R)
            xyzT_sb = consts.tile([3, M], F32R)
            bqk_sb = consts.tile([128, 2], F32)
            bv_sb = consts.tile([128, 1], F32)
            id_sb = consts.tile([128, 128], F32R)
            eps_sb = consts.tile([128, 1], F32)
            nc.sync.dma_start(out=wqk_sb, in_=wqk.rearrange("(k p) n -> p k n", p=128))
            nc.sync.dma_start(out=wv_sb, in_=wv.rearrange("(k p) n -> p k n", p=128))
            nc.sync.dma_start(out=wsp_sb, in_=wsp[:, :])
            nc.sync.dma_start(out=xyzT_sb, in_=xyzT[:, :])
            nc.sync.dma_start(out=bqk_sb, in_=bqk[:, :])
            nc.sync.dma_start(out=bv_sb, in_=bv[:, :])
            nc.sync.dma_start(out=id_sb, in_=idm[:, :])
            nc.vector.memset(eps_sb, EPS)

            x_sb = xall.tile([128, MT, DIM], F32)
            nc.sync.dma_start(out=x_sb, in_=x.rearrange("(t p) d -> p t d", p=128))

            # --- layernorm statistics for all 16 m-tiles ---
            st = stats.tile([128, MT, 6], F32)
            mv = stats.tile([128, MT, 2], F32)
            for t in range(MT):
                nc.vector.bn_stats(out=st[:, t, :], in_=x_sb[:, t, :])
                nc.vector.bn_aggr(out=mv[:, t, :], in_=st[:, t, :])
            rstd = stats.tile([128, MT], F32)
            nmr = stats.tile([128, MT], F32)
            nc.scalar.activation(rstd, mv[:, :, 1], SQRT, bias=eps_sb[:, 0:1])
            nc.vector.reciprocal(rstd, rstd)
            nc.vector.tensor_tensor(out=nmr, in0=mv[:, :, 0], in1=rstd, op=MULT)
            nc.vector.tensor_scalar(nmr, nmr, -1.0, None, op0=MULT)

            zt_sb = ztp.tile([128, KT, M], F32R)
            sb2 = qkvsb.tile([128, M], F32)
            qT = qkvsb.tile([128, M], F32R)
            kT = qkvsb.tile([128, M], F32R)
            vT = qkvsb.tile([128, M], F32R)
            vp_sb = qkvsb.tile([128, MT, 130], F32R)
            ones16 = qkvsb.tile([128, MT], F32)
            nc.vector.memset(ones16, 1.0)
            nc.vector.tensor_copy(vp_sb[:, :, 64], ones16[:, :])
            nc.vector.tensor_copy(vp_sb[:, :, 129], ones16[:, :])

            with tc.tile_pool(name="tps", bufs=3, space="PSUM") as tps, \
                 tc.tile_pool(name="qkps", bufs=3, space="PSUM") as qkps:
                # --- standardize (gpsimd) + transpose z tiles (PE) ---
                for t in range(MT):
                    z = zp.tile([128, DIM], F32)
                    nc.gpsimd.tensor_scalar(z, x_sb[:, t, :], rstd[:, t:t + 1],
                                            nmr[:, t:t + 1], op0=MULT, op1=ADD)
                    for k in range(KT):
                        pt = tps.tile([128, 128], F32, tag="tp")
                        nc.tensor.transpose(pt[:, :], z[:, 128 * k:128 * (k + 1)], id_sb[:, :].bitcast(F32))
                        nc.vector.tensor_copy(zt_sb[:, k, 128 * t:128 * (t + 1)], pt[:, :])

                # --- q,k,v projections (transposed layout [d, m]) ---
                for c in range(NCH):
                    cs = slice(512 * c, 512 * (c + 1))
                    pq = qkps.tile([128, 512], F32, tag="qk")
                    for k in range(KT):
                        nc.tensor.matmul(pq[:, :], wqk_sb[:, k, 0:128], zt_sb[:, k, cs],
                                         start=(k == 0), stop=(k == KT - 1))
                    nc.vector.tensor_scalar(qT[:, cs], pq[:, :], bqk_sb[:, 0:1], None, op0=ADD)
                    pk = qkps.tile([128, 512], F32, tag="qk")
                    for k in range(KT):
                        nc.tensor.matmul(pk[:, :], wqk_sb[:, k, 128:256], zt_sb[:, k, cs],
                                         start=(k == 0), stop=(k == KT - 1))
                    nc.vector.tensor_scalar(kT[:, cs], pk[:, :], bqk_sb[:, 1:2], None, op0=ADD)
                    # s_b^T = (xyz @ w_sp)^T, duplicated into both head halves
                    psb = qkps.tile([128, 512], F32, tag="qk")
                    nc.tensor.matmul(psb[0:64, :], wsp_sb[:, :], xyzT_sb[:, cs],
                                     start=True, stop=True)
                    nc.vector.tensor_copy(sb2[0:64, cs], psb[0:64, :])
                    nc.vector.tensor_copy(sb2[64:128, cs], psb[0:64, :])
                    pv = qkps.tile([128, 512], F32, tag="qk")
                    for k in range(KT):
                        nc.tensor.matmul(pv[:, :], wv_sb[:, k, :], zt_sb[:, k, cs],
                                         start=(k == 0), stop=(k == KT - 1))
                    # v' = v + bias_v + s_b  (one fused DVE op)
                    nc.vector.scalar_tensor_tensor(vT[:, cs], pv[:, :], bv_sb[:, 0:1],
                                                   sb2[:, cs], op0=ADD, op1=ADD)

                # --- transpose v' back to natural [j, d] layout with ones cols ---
                for t in range(MT):
                    pt = tps.tile([128, 128], F32R, tag="tp")
                    nc.tensor.transpose(pt[:, :], vT[:, 128 * t:128 * (t + 1)],
                                        id_sb[:, :])
                    nc.vector.tensor_copy(vp_sb[:, t, 0:64], pt[0:128, 0:64])
                    nc.vector.tensor_copy(vp_sb[:, t, 65:129], pt[0:128, 64:128])

            # --- attention: S^T tiles -> exp -> accumulate (P@v')^T ---
            ot_sb = otp.tile([128, M], F32)
            with tc.tile_pool(name="sab", bufs=2, space="PSUM") as sabp, \
                 tc.tile_pool(name="ops", bufs=2, space="PSUM") as ops:
                for c in range(NCH):
                    cs = slice(512 * c, 512 * (c + 1))
                    oa = ops.tile([128, 512], F32, tag="oa")
                    ob = ops.tile([128, 512], F32, tag="ob")
                    for j in range(MT):
                        js = slice(128 * j, 128 * (j + 1))
                        sab = sabp.tile([128, 1024], F32, tag="sab")
                        nc.tensor.matmul(sab[:, 0:512], kT[0:64, js], qT[0:64, cs],
                                         start=True, stop=True)
                        nc.tensor.matmul(sab[:, 512:1024], kT[64:128, js], qT[64:128, cs],
                                         start=True, stop=True)
                        e = esb.tile([128, 1024], F32R, tag="e")
                        nc.scalar.activation(e, sab[:, :], EXP, scale=SCALE)
                        nc.tensor.matmul(oa[0:65, :], vp_sb[:, j, 0:65], e[:, 0:512],
                                         start=(j == 0), stop=(j == MT - 1),
                                         skip_group_check=True)
                        nc.tensor.matmul(ob[0:65, :], vp_sb[:, j, 65:130], e[:, 512:1024],
                                         start=(j == 0), stop=(j == MT - 1),
                                         skip_group_check=True)
                    ra = small.tile([1, 512], F32, tag="r")
                    rb = small.tile([1, 512], F32, tag="r")
                    nc.vector.reciprocal(ra, oa[64:65, :])
                    nc.vector.reciprocal(rb, ob[64:65, :])
                    rab = small.tile([64, 512], F32, tag="rb")
                    rbb = small.tile([64, 512], F32, tag="rb")
                    nc.gpsimd.partition_broadcast(rab, ra[0:1, :])
                    nc.gpsimd.partition_broadcast(rbb, rb[0:1, :])
                    nc.vector.tensor_tensor(out=ot_sb[0:64, cs], in0=oa[0:64, :], in1=rab, op=MULT)
                    nc.vector.tensor_tensor(out=ot_sb[64:128, cs], in0=ob[0:64, :], in1=rbb, op=MULT)
                    nc.sync.dma_start(out=otn[:, cs], in_=ot_sb[:, cs])
    nc.compile()
    return nc


def _build_launch2() -> bass.Bass:
    """Per core: 512 token rows.  y^T = gelu(w_out^T @ ot - wfold^T @ s_b^T
    + b_out) + x^T, all in [dim, i] layout."""
    nc = bacc.Bacc()
    ot = nc.dram_tensor("ot", [INNER, 512], F32R, kind="ExternalInput")
    wout = nc.dram_tensor("wout", [INNER, DIM], F32R, kind="ExternalInput")
    wfold = nc.dram_tensor("wfold", [DH, DIM], F32R, kind="ExternalInput")
    xyzTs = nc.dram_tensor("xyzTs", [3, 512], F32R, kind="ExternalInput")
    wsp = nc.dram_tensor("wsp", [3, DH], F32R, kind="ExternalInput")
    bout = nc.dram_tensor("bout", [128, KT], F32, kind="ExternalInput")
    xT = nc.dram_tensor("xT", [DIM, 512], F32, kind="ExternalInput")
    y = nc.dram_tensor("y", [DIM, 512], F32, kind="ExternalOutput")

    with tile.TileContext(nc) as tc:
        with tc.tile_pool(name="sb", bufs=1) as sb, \
             tc.tile_pool(name="ps", bufs=2, space="PSUM") as ps, \
             tc.tile_pool(name="ps1", bufs=1, space="PSUM") as ps1:
            ot_sb = sb.tile([128, KT, 512], F32R)
            wout_sb = sb.tile([128, KT, DIM], F32R)
            wfold_sb = sb.tile([DH, DIM], F32R)
            xyzT_sb = sb.tile([3, 512], F32R)
            wsp_sb = sb.tile([3, DH], F32R)
            bout_sb = sb.tile([128, KT], F32)
            xT_sb = sb.tile([128, KT, 512], F32)
            y_sb = sb.tile([128, KT, 512], F32)
            nc.sync.dma_start(out=ot_sb, in_=ot.rearrange("(k p) n -> p k n", p=128))
            nc.sync.dma_start(out=wout_sb, in_=wout.rearrange("(k p) n -> p k n", p=128))
            nc.sync.dma_start(out=wfold_sb, in_=wfold[:, :])
            nc.sync.dma_start(out=xyzT_sb, in_=xyzTs[:, :])
            nc.sync.dma_start(out=wsp_sb, in_=wsp[:, :])
            nc.sync.dma_start(out=bout_sb, in_=bout[:, :])
            nc.sync.dma_start(out=xT_sb, in_=xT.rearrange("(k p) n -> p k n", p=128))

            sbt_ps = ps1.tile([128, 512], F32)
            nc.tensor.matmul(sbt_ps[0:DH, :], wsp_sb[:, :], xyzT_sb[:, :],
                             start=True, stop=True)
            sbt = sb.tile([DH, 512], F32R)
            nc.vector.tensor_copy(sbt, sbt_ps[0:DH, :])

            for d in range(KT):
                ds = slice(128 * d, 128 * (d + 1))
                py = ps.tile([128, 512], F32, tag="y")
                for k in range(KT):
                    nc.tensor.matmul(py[:, :], wout_sb[:, k, ds], ot_sb[:, k, :],
                                     start=(k == 0), stop=False, skip_group_check=True)
                nc.tensor.matmul(py[:, :], wfold_sb[:, ds], sbt[:, :],
                                 start=False, stop=True, skip_group_check=True)
                g = sb.tile([128, 512], F32)
                nc.scalar.activation(g, py[:, :], GELU, bias=bout_sb[:, d:d + 1])
                nc.vector.tensor_tensor(out=y_sb[:, d, :], in0=g, in1=xT_sb[:, d, :], op=ADD)
            nc.sync.dma_start(out=y.rearrange("(k p) n -> p k n", p=128), in_=y_sb)
    nc.compile()
    return nc


_NC1 = None
_NC2 = None


def prepare_inputs1(xyzs, features, ln_gamma, ln_beta, w_qkv, w_sp, w_out, b_out):
    wp = ln_gamma[:, None] * w_qkv                      # [512, 1536]
    bias_full = ln_beta @ w_qkv                         # [1536]
    idm = np.eye(128, dtype=np.float32)
    xf = features.reshape(B, M, DIM)
    xyzf = xyzs.reshape(B, M, 3)
    in1 = []
    for c in range(NCORES):
        b, p = divmod(c, 4)
        h0 = 2 * p
        qcols = [wp[:, 64 * h:64 * (h + 1)] for h in (h0, h0 + 1)]
        kcols = [wp[:, INNER + 64 * h:INNER + 64 * (h + 1)] for h in (h0, h0 + 1)]
        vcols = [wp[:, 2 * INNER + 64 * h:2 * INNER + 64 * (h + 1)] for h in (h0, h0 + 1)]
        bq = np.concatenate([bias_full[64 * h:64 * (h + 1)] for h in (h0, h0 + 1)])
        bk = np.concatenate([bias_full[INNER + 64 * h:INNER + 64 * (h + 1)]
                             for h in (h0, h0 + 1)])
        bvv = np.concatenate([bias_full[2 * INNER + 64 * h:2 * INNER + 64 * (h + 1)]
                              for h in (h0, h0 + 1)])
        in1.append({
            "x": np.ascontiguousarray(xf[b]),
            "xyzT": np.ascontiguousarray(xyzf[b].T),
            "wqk": np.ascontiguousarray(np.concatenate(qcols + kcols, axis=1)),
            "wv": np.ascontiguousarray(np.concatenate(vcols, axis=1)),
            "wsp": np.ascontiguousarray(w_sp),
            "bqk": np.ascontiguousarray(np.stack([bq, bk], axis=1)),
            "bv": np.ascontiguousarray(bvv[:, None]),
            "idm": idm,
        })
    return in1


def assemble_mid(results1):
    otf = np.empty((B, INNER, M), dtype=np.float32)
    for c in range(NCORES):
        b, p = divmod(c, 4)
        otf[b, 128 * p:128 * (p + 1)] = results1[c]["otn"]
    return otf


def prepare_inputs2(xyzs, features, ln_gamma, ln_beta, w_qkv, w_sp, w_out, b_out,
                    otf):
    xf = features.reshape(B, M, DIM)
    xyzf = xyzs.reshape(B, M, 3)
    wfold = -w_out.reshape(H, DH, DIM).sum(axis=0)      # [64, 512]
    in2 = []
    for c in range(NCORES):
        b, p = divmod(c, 4)
        isl = slice(512 * p, 512 * (p + 1))
        in2.append({
            "ot": np.ascontiguousarray(otf[b][:, isl]),
            "wout": np.ascontiguousarray(w_out),
            "wfold": np.ascontiguousarray(wfold),
            "xyzTs": np.ascontiguousarray(xyzf[b].T[:, isl]),
            "wsp": np.ascontiguousarray(w_sp),
            "bout": np.ascontiguousarray(b_out.reshape(KT, 128).T),
            "xT": np.ascontiguousarray(xf[b].T[:, isl]),
        })
    return in2


def assemble_out(results2):
    out = np.empty((B, M, DIM), dtype=np.float32)
    for c in range(NCORES):
        b, p = divmod(c, 4)
        out[b, 512 * p:512 * (p + 1)] = results2[c]["y"].T
    return out.reshape(B, L, N, DIM)


_NC1 = None
_NC2 = None


def get_nc1():
    global _NC1
    if _NC1 is None:
        _NC1 = _build_launch1()
    return _NC1


def get_nc2():
    global _NC2
    if _NC2 is None:
        _NC2 = _build_launch2()
    return _NC2


def kernel(xyzs, features, ln_gamma, ln_beta, w_qkv, w_sp, w_out, b_out):
    args = [np.asarray(a, dtype=np.float32) for a in
            (xyzs, features, ln_gamma, ln_beta, w_qkv, w_sp, w_out, b_out)]
    in1 = prepare_inputs1(*args)
    r1 = run_bass_kernel_spmd(get_nc1(), in1, core_ids=list(range(NCORES)))
    otf = assemble_mid(r1.results)
    in2 = prepare_inputs2(*args, otf)
    r2 = run_bass_kernel_spmd(get_nc2(), in2, core_ids=list(range(NCORES)))
    return assemble_out(r2.results)


# revision 6
# speedup vs baseline: 143.7009x; 143.7009x over previous
"""Trainium2 Bass kernel for nn_Attention_695784702572.

Model (per batch b, head h):
    z      = layernorm(features)                      (standardize; gamma/beta folded into weights)
    q,k,v  = z @ Wq, z @ Wk, z @ Wv                   (per-head slices of w_qkv)
    S      = q @ k^T * DH^-0.5
    P      = softmax(S)   (rows sum to 1)
    agg    = P @ delta = P @ xyz - xyz                (since softmax rows sum to 1)
    out_h  = P @ v + (P @ xyz - xyz) @ w_sp
           = P @ (v + xyz @ w_sp) - xyz @ w_sp
    y      = gelu(concat_h(out_h) @ w_out + b_out) + features

Sharding: 16 (b,h) pairs over 8 cores -> core c handles batch c//4, heads
2*(c%4), 2*(c%4)+1 for the attention launch.  The final projection mixes
heads, so a second SPMD launch row-shards the 4096 token rows 512/core.
The "- xyz @ w_sp" term is identical for every head so it is folded into
the final projection as  - s_b @ (sum_h w_out[64h:64h+64])  (exact algebra).

All matmuls run as fp32r (TF32-like, full PE rate at N>=256); softmax skips
the max-subtraction (|S*scale| < ~6 for these inputs, exp is safe in fp32);
the softmax row-sum comes for free from a ones-column appended to v.
"""

import numpy as np

import concourse.bacc as bacc
import concourse.bass as bass
import concourse.tile as tile
from concourse import mybir
from concourse.bass_utils import run_bass_kernel_spmd

F32 = mybir.dt.float32
F32R = mybir.dt.float32r
EXP = mybir.ActivationFunctionType.Exp
GELU = mybir.ActivationFunctionType.Gelu
SQRT = mybir.ActivationFunctionType.Sqrt
MULT = mybir.AluOpType.mult
ADD = mybir.AluOpType.add

B, L, N, DIM = 2, 8, 256, 512
M = L * N            # 2048 tokens per batch
H, DH = 8, 64
INNER = H * DH       # 512
EPS = 1e-5
SCALE = DH ** -0.5
NCORES = 8
MT = M // 128        # 16 m-tiles
KT = DIM // 128      # 4 contraction tiles
NCH = M // 512       # 4 i-chunks


def _build_launch1(repeat: int = 1) -> bass.Bass:
    """Per core: batch b, 2 heads. In: x[2048,512], xyzT[3,2048], weights.
    Out: otn[128,2048] = normalized (P@v')^T for the 2 heads (rows 0-63 head
    A, 64-127 head B), where v' = v + xyz@w_sp."""
    nc = bacc.Bacc()
    x = nc.dram_tensor("x", [M, DIM], F32, kind="ExternalInput")
    xyzT = nc.dram_tensor("xyzT", [3, M], F32R, kind="ExternalInput")
    wqk = nc.dram_tensor("wqk", [DIM, 256], F32R, kind="ExternalInput")
    wv = nc.dram_tensor("wv", [DIM, 128], F32R, kind="ExternalInput")
    wsp = nc.dram_tensor("wsp", [3, DH], F32R, kind="ExternalInput")
    bqk = nc.dram_tensor("bqk", [128, 2], F32, kind="ExternalInput")
    bv = nc.dram_tensor("bv", [128, 1], F32, kind="ExternalInput")
    idm = nc.dram_tensor("idm", [128, 128], F32R, kind="ExternalInput")
    otn = nc.dram_tensor("otn", [128, M], F32, kind="ExternalOutput")

    with tile.TileContext(nc) as tc:
      for _rep in range(repeat):
        with tc.tile_pool(name="consts", bufs=1) as consts, \
             tc.tile_pool(name="xall", bufs=1) as xall, \
             tc.tile_pool(name="stats", bufs=1) as stats, \
             tc.tile_pool(name="zp", bufs=3) as zp, \
             tc.tile_pool(name="ztp", bufs=1) as ztp, \
             tc.tile_pool(name="qkvsb", bufs=1) as qkvsb, \
             tc.tile_pool(name="esb", bufs=3) as esb, \
             tc.tile_pool(name="otp", bufs=1) as otp, \
             tc.tile_pool(name="small", bufs=4) as small:

            wqk_sb = consts.tile([128, KT, 256], F32R)
            wv_sb = consts.tile([128, KT, 128], F32R)
            wsp_sb = consts.tile([3, DH], F32R)
            xyzT_sb = consts.tile([3, M], F32R)
            bqk_sb = consts.tile([128, 2], F32)
            bv_sb = consts.tile([128, 1], F32)
            id_sb = consts.tile([128, 128], F32R)
            eps_sb = consts.tile([128, 1], F32)
            nc.sync.dma_start(out=wqk_sb, in_=wqk.rearrange("(k p) n -> p k n", p=128))
            nc.sync.dma_start(out=wv_sb, in_=wv.rearrange("(k p) n -> p k n", p=128))
            nc.sync.dma_start(out=wsp_sb, in_=wsp[:, :])
            nc.sync.dma_start(out=xyzT_sb, in_=xyzT[:, :])
            nc.sync.dma_start(out=bqk_sb, in_=bqk[:, :])
            nc.sync.dma_start(out=bv_sb, in_=bv[:, :])
            nc.sync.dma_start(out=id_sb, in_=idm[:, :])
            nc.vector.memset(eps_sb, EPS)

            x_sb = xall.tile([128, MT, DIM], F32)
            nc.sync.dma_start(out=x_sb, in_=x.rearrange("(t p) d -> p t d", p=128))

            # --- layernorm statistics for all 16 m-tiles ---
            st = stats.tile([128, MT, 6], F32)
            mv = stats.tile([128, MT, 2], F32)
            for t in range(MT):
                nc.vector.bn_stats(out=st[:, t, :], in_=x_sb[:, t, :])
                nc.vector.bn_aggr(out=mv[:, t, :], in_=st[:, t, :])
            rstd = stats.tile([128, MT], F32)
            nmr = stats.tile([128, MT], F32)
            nc.scalar.activation(rstd, mv[:, :, 1], SQRT, bias=eps_sb[:, 0:1])
            nc.vector.reciprocal(rstd, rstd)
            nc.vector.tensor_tensor(out=nmr, in0=mv[:, :, 0], in1=rstd, op=MULT)
            nc.vector.tensor_scalar(nmr, nmr, -1.0, None, op0=MULT)

            zt_sb = ztp.tile([128, KT, M], F32R)
            sb2 = qkvsb.tile([128, M], F32)
            qT = qkvsb.tile([128, M], F32R)
            kT = qkvsb.tile([128, M], F32R)
            vT = qkvsb.tile([128, M], F32R)
            vp_sb = qkvsb.tile([128, MT, 130], F32R)
            ones16 = qkvsb.tile([128, MT], F32)
            nc.vector.memset(ones16, 1.0)
            nc.vector.tensor_copy(vp_sb[:, :, 64], ones16[:, :])
            nc.vector.tensor_copy(vp_sb[:, :, 129], ones16[:, :])

            with tc.tile_pool(name="tps", bufs=3, space="PSUM") as tps, \
                 tc.tile_pool(name="qkps", bufs=3, space="PSUM") as qkps:
                # --- standardize (gpsimd) + transpose z tiles (PE) ---
                for t in range(MT):
                    z = zp.tile([128, DIM], F32)
                    nc.gpsimd.tensor_scalar(z, x_sb[:, t, :], rstd[:, t:t + 1],
                                            nmr[:, t:t + 1], op0=MULT, op1=ADD)
                    for k in range(KT):
                        pt = tps.tile([128, 128], F32, tag="tp")
                        nc.tensor.transpose(pt[:, :], z[:, 128 * k:128 * (k + 1)], id_sb[:, :].bitcast(F32))
                        nc.vector.tensor_copy(zt_sb[:, k, 128 * t:128 * (t + 1)], pt[:, :])

                # --- q,k,v projections (transposed layout [d, m]) ---
                for c in range(NCH):
                    cs = slice(512 * c, 512 * (c + 1))
                    pq = qkps.tile([128, 512], F32, tag="qk")
                    for k in range(KT):
                        nc.tensor.matmul(pq[:, :], wqk_sb[:, k, 0:128], zt_sb[:, k, cs],
                                         start=(k == 0), stop=(k == KT - 1))
                    nc.vector.tensor_scalar(qT[:, cs], pq[:, :], bqk_sb[:, 0:1], None, op0=ADD)
                    pk = qkps.tile([128, 512], F32, tag="qk")
                    for k in range(KT):
                        nc.tensor.matmul(pk[:, :], wqk_sb[:, k, 128:256], zt_sb[:, k, cs],
                                         start=(k == 0), stop=(k == KT - 1))
                    nc.vector.tensor_scalar(kT[:, cs], pk[:, :], bqk_sb[:, 1:2], None, op0=ADD)
                    # s_b^T = (xyz @ w_sp)^T, duplicated into both head halves
                    psb = qkps.tile([128, 512], F32, tag="qk")
                    nc.tensor.matmul(psb[0:64, :], wsp_sb[:, :], xyzT_sb[:, cs],
                                     start=True, stop=True)
                    nc.vector.tensor_copy(sb2[0:64, cs], psb[0:64, :])
                    nc.vector.tensor_copy(sb2[64:128, cs], psb[0:64, :])
                    pv = qkps.tile([128, 512], F32, tag="qk")
                    for k in range(KT):
                        nc.tensor.matmul(pv[:, :], wv_sb[:, k, :], zt_sb[:, k, cs],
                                         start=(k == 0), stop=(k == KT - 1))
                    # v' = v + bias_v + s_b  (one fused DVE op)
                    nc.vector.scalar_tensor_tensor(vT[:, cs], pv[:, :], bv_sb[:, 0:1],
                                                   sb2[:, cs], op0=ADD, op1=ADD)

                # --- transpose v' back to natural [j, d] layout with ones cols ---
                for t in range(MT):
                    pt = tps.tile([128, 128], F32R, tag="tp")
                    nc.tensor.transpose(pt[:, :], vT[:, 128 * t:128 * (t + 1)],
                                        id_sb[:, :])
                    nc.vector.tensor_copy(vp_sb[:, t, 0:64], pt[0:128, 0:64])
                    nc.vector.tensor_copy(vp_sb[:, t, 65:129], pt[0:128, 64:128])

            # --- attention: S^T tiles -> exp -> accumulate (P@v')^T ---
            ot_sb = otp.tile([128, M], F32)
            with tc.tile_pool(name="sab", bufs=2, space="PSUM") as sabp, \
                 tc.tile_pool(name="ops", bufs=2, space="PSUM") as ops:
                for c in range(NCH):
                    cs = slice(512 * c, 512 * (c + 1))
                    oa = ops.tile([128, 512], F32, tag="oa")
                    ob = ops.tile([128, 512], F32, tag="ob")
                    for j in range(MT):
                        js = slice(128 * j, 128 * (j + 1))
                        sab = sabp.tile([128, 1024], F32, tag="sab")
                        nc.tensor.matmul(sab[:, 0:512], kT[0:64, js], qT[0:64, cs],
                                         start=True, stop=True)
                        nc.tensor.matmul(sab[:, 512:1024], kT[64:128, js], qT[64:128, cs],
                                         start=True, stop=True)
                        e = esb.tile([128, 1024], F32R, tag="e")
                        nc.scalar.activation(e, sab[:, :], EXP, scale=SCALE)
                        nc.tensor.matmul(oa[0:65, :], vp_sb[:, j, 0:65], e[:, 0:512],
                                         start=(j == 0), stop=(j == MT - 1),
                                         skip_group_check=True)
                        nc.tensor.matmul(ob[0:65, :], vp_sb[:, j, 65:130], e[:, 512:1024],
                                         start=(j == 0), stop=(j == MT - 1),
                                         skip_group_check=True)
                    ra = small.tile([1, 512], F32, tag="r")
                    rb = small.tile([1, 512], F32, tag="r")
                    nc.vector.reciprocal(ra, oa[64:65, :])
                    nc.vector.reciprocal(rb, ob[64:65, :])
                    rab = small.tile([64, 512], F32, tag="rb")
                    rbb = small.tile([64, 512], F32, tag="rb")
                    nc.gpsimd.partition_broadcast(rab, ra[0:1, :])
                    nc.gpsimd.partition_broadcast(rbb, rb[0:1, :])
                    nc.vector.tensor_tensor(out=ot_sb[0:64, cs], in0=oa[0:64, :], in1=rab, op=MULT)
                    nc.vector.tensor_tensor(out=ot_sb[64:128, cs], in0=ob[0:64, :], in1=rbb, op=MULT)
                    nc.sync.dma_start(out=otn[:, cs], in_=ot_sb[:, cs])
    nc.compile()
    return nc


def _build_launch2(repeat: int = 1) -> bass.Bass:
    """Per core: 512 token rows.  y^T = gelu(w_out^T @ ot - wfold^T @ s_b^T
    + b_out) + x^T, all in [dim, i] layout."""
    nc = bacc.Bacc()
    ot = nc.dram_tensor("ot", [INNER, 512], F32R, kind="ExternalInput")
    wout = nc.dram_tensor("wout", [INNER, DIM], F32R, kind="ExternalInput")
    wfold = nc.dram_tensor("wfold", [DH, DIM], F32R, kind="ExternalInput")
    xyzTs = nc.dram_tensor("xyzTs", [3, 512], F32R, kind="ExternalInput")
    wsp = nc.dram_tensor("wsp", [3, DH], F32R, kind="ExternalInput")
    bout = nc.dram_tensor("bout", [128, KT], F32, kind="ExternalInput")
    xT = nc.dram_tensor("xT", [DIM, 512], F32, kind="ExternalInput")
    y = nc.dram_tensor("y", [DIM, 512], F32, kind="ExternalOutput")

    with tile.TileContext(nc) as tc:
      for _rep in range(repeat):
        with tc.tile_pool(name="sb", bufs=1) as sb, \
             tc.tile_pool(name="ps", bufs=2, space="PSUM") as ps, \
             tc.tile_pool(name="ps1", bufs=1, space="PSUM") as ps1:
            ot_sb = sb.tile([128, KT, 512], F32R)
            wout_sb = sb.tile([128, KT, DIM], F32R)
            wfold_sb = sb.tile([DH, DIM], F32R)
            xyzT_sb = sb.tile([3, 512], F32R)
            wsp_sb = sb.tile([3, DH], F32R)
            bout_sb = sb.tile([128, KT], F32)
            xT_sb = sb.tile([128, KT, 512], F32)
            y_sb = sb.tile([128, KT, 512], F32)
            nc.sync.dma_start(out=ot_sb, in_=ot.rearrange("(k p) n -> p k n", p=128))
            nc.sync.dma_start(out=wout_sb, in_=wout.rearrange("(k p) n -> p k n", p=128))
            nc.sync.dma_start(out=wfold_sb, in_=wfold[:, :])
            nc.sync.dma_start(out=xyzT_sb, in_=xyzTs[:, :])
            nc.sync.dma_start(out=wsp_sb, in_=wsp[:, :])
            nc.sync.dma_start(out=bout_sb, in_=bout[:, :])
            nc.sync.dma_start(out=xT_sb, in_=xT.rearrange("(k p) n -> p k n", p=128))

            sbt_ps = ps1.tile([128, 512], F32)
            nc.tensor.matmul(sbt_ps[0:DH, :], wsp_sb[:, :], xyzT_sb[:, :],
                             start=True, stop=True)
            sbt = sb.tile([DH, 512], F32R)
            nc.vector.tensor_copy(sbt, sbt_ps[0:DH, :])

            for d in range(KT):
                ds = slice(128 * d, 128 * (d + 1))
                py = ps.tile([128, 512], F32, tag="y")
                for k in range(KT):
                    nc.tensor.matmul(py[:, :], wout_sb[:, k, ds], ot_sb[:, k, :],
                                     start=(k == 0), stop=False, skip_group_check=True)
                nc.tensor.matmul(py[:, :], wfold_sb[:, ds], sbt[:, :],
                                 start=False, stop=True, skip_group_check=True)
                g = sb.tile([128, 512], F32)
                nc.scalar.activation(g, py[:, :], GELU, bias=bout_sb[:, d:d + 1])
                nc.vector.tensor_tensor(out=y_sb[:, d, :], in0=g, in1=xT_sb[:, d, :], op=ADD)
            nc.sync.dma_start(out=y.rearrange("(k p) n -> p k n", p=128), in_=y_sb)
    nc.compile()
    return nc


_NC1 = None
_NC2 = None


def prepare_inputs1(xyzs, features, ln_gamma, ln_beta, w_qkv, w_sp, w_out, b_out):
    wp = ln_gamma[:, None] * w_qkv                      # [512, 1536]
    bias_full = ln_beta @ w_qkv                         # [1536]
    idm = np.eye(128, dtype=np.float32)
    xf = features.reshape(B, M, DIM)
    xyzf = xyzs.reshape(B, M, 3)
    in1 = []
    for c in range(NCORES):
        b, p = divmod(c, 4)
        h0 = 2 * p
        qcols = [wp[:, 64 * h:64 * (h + 1)] for h in (h0, h0 + 1)]
        kcols = [wp[:, INNER + 64 * h:INNER + 64 * (h + 1)] for h in (h0, h0 + 1)]
        vcols = [wp[:, 2 * INNER + 64 * h:2 * INNER + 64 * (h + 1)] for h in (h0, h0 + 1)]
        bq = np.concatenate([bias_full[64 * h:64 * (h + 1)] for h in (h0, h0 + 1)])
        bk = np.concatenate([bias_full[INNER + 64 * h:INNER + 64 * (h + 1)]
                             for h in (h0, h0 + 1)])
        bvv = np.concatenate([bias_full[2 * INNER + 64 * h:2 * INNER + 64 * (h + 1)]
                              for h in (h0, h0 + 1)])
        in1.append({
            "x": np.ascontiguousarray(xf[b]),
            "xyzT": np.ascontiguousarray(xyzf[b].T),
            "wqk": np.ascontiguousarray(np.concatenate(qcols + kcols, axis=1)),
            "wv": np.ascontiguousarray(np.concatenate(vcols, axis=1)),
            "wsp": np.ascontiguousarray(w_sp),
            "bqk": np.ascontiguousarray(np.stack([bq, bk], axis=1)),
            "bv": np.ascontiguousarray(bvv[:, None]),
            "idm": idm,
        })
    return in1


def assemble_mid(results1):
    otf = np.empty((B, INNER, M), dtype=np.float32)
    for c in range(NCORES):
        b, p = divmod(c, 4)
        otf[b, 128 * p:128 * (p + 1)] = results1[c]["otn"]
    return otf


def prepare_inputs2(xyzs, features, ln_gamma, ln_beta, w_qkv, w_sp, w_out, b_out,
                    otf):
    xf = features.reshape(B, M, DIM)
    xyzf = xyzs.reshape(B, M, 3)
    wfold = -w_out.reshape(H, DH, DIM).sum(axis=0)      # [64, 512]
    in2 = []
    for c in range(NCORES):
        b, p = divmod(c, 4)
        isl = slice(512 * p, 512 * (p + 1))
        in2.append({
            "ot": np.ascontiguousarray(otf[b][:, isl]),
            "wout": np.ascontiguousarray(w_out),
            "wfold": np.ascontiguousarray(wfold),
            "xyzTs": np.ascontiguousarray(xyzf[b].T[:, isl]),
            "wsp": np.ascontiguousarray(w_sp),
            "bout": np.ascontiguousarray(b_out.reshape(KT, 128).T),
            "xT": np.ascontiguousarray(xf[b].T[:, isl]),
        })
    return in2


def assemble_out(results2):
    out = np.empty((B, M, DIM), dtype=np.float32)
    for c in range(NCORES):
        b, p = divmod(c, 4)
        out[b, 512 * p:512 * (p + 1)] = results2[c]["y"].T
    return out.reshape(B, L, N, DIM)


_NC1 = None
_NC2 = None


def get_nc1():
    global _NC1
    if _NC1 is None:
        _NC1 = _build_launch1()
    return _NC1


def get_nc2():
    global _NC2
    if _NC2 is None:
        _NC2 = _build_launch2()
    return _NC2


def kernel(xyzs, features, ln_gamma, ln_beta, w_qkv, w_sp, w_out, b_out):
    args = [np.asarray(a, dtype=np.float32) for a in
            (xyzs, features, ln_gamma, ln_beta, w_qkv, w_sp, w_out, b_out)]
    in1 = prepare_inputs1(*args)
    r1 = run_bass_kernel_spmd(get_nc1(), in1, core_ids=list(range(NCORES)))
    otf = assemble_mid(r1.results)
    in2 = prepare_inputs2(*args, otf)
    r2 = run_bass_kernel_spmd(get_nc2(), in2, core_ids=list(range(NCORES)))
    return assemble_out(r2.results)


# revision 7
# speedup vs baseline: 312.9015x; 2.1775x over previous
"""Trainium2 Bass kernel for nn_Attention_695784702572.

Model (per batch b, head h):
    z      = layernorm(features)                      (standardize; gamma/beta folded into weights)
    q,k,v  = z @ Wq, z @ Wk, z @ Wv                   (per-head slices of w_qkv)
    S      = q @ k^T * DH^-0.5
    P      = softmax(S)   (rows sum to 1)
    agg    = P @ delta = P @ xyz - xyz                (since softmax rows sum to 1)
    out_h  = P @ v + (P @ xyz - xyz) @ w_sp
           = P @ (v + xyz @ w_sp) - xyz @ w_sp
    y      = gelu(concat_h(out_h) @ w_out + b_out) + features

Sharding: 16 (b,h) pairs over 8 cores -> core c handles batch c//4, heads
2*(c%4), 2*(c%4)+1 for the attention launch.  The final projection mixes
heads, so a second SPMD launch row-shards the 4096 token rows 512/core.
The "- xyz @ w_sp" term is identical for every head so it is folded into
the final projection as  - s_b @ (sum_h w_out[64h:64h+64])  (exact algebra).

All matmuls run as fp32r (TF32-like, full PE rate at N>=256); softmax skips
the max-subtraction (|S*scale| < ~6 for these inputs, exp is safe in fp32);
the softmax row-sum comes for free from a ones-column appended to v.
"""

import numpy as np

import concourse.bacc as bacc
import concourse.bass as bass
import concourse.tile as tile
from concourse import mybir
from concourse.bass_utils import run_bass_kernel_spmd

F32 = mybir.dt.float32
F32R = mybir.dt.float32r
EXP = mybir.ActivationFunctionType.Exp
GELU = mybir.ActivationFunctionType.Gelu
SQRT = mybir.ActivationFunctionType.Sqrt
MULT = mybir.AluOpType.mult
ADD = mybir.AluOpType.add

B, L, N, DIM = 2, 8, 256, 512
M = L * N            # 2048 tokens per batch
H, DH = 8, 64
INNER = H * DH       # 512
EPS = 1e-5
SCALE = DH ** -0.5
NCORES = 8
MT = M // 128        # 16 m-tiles
KT = DIM // 128      # 4 contraction tiles
NCH = M // 512       # 4 i-chunks


def _build_launch1(repeat: int = 1) -> bass.Bass:
    """Per core: batch b, 2 heads. In: x[2048,512], xyzT[3,2048], weights.
    Out: otn[128,2048] = normalized (P@v')^T for the 2 heads (rows 0-63 head
    A, 64-127 head B), where v' = v + xyz@w_sp."""
    nc = bacc.Bacc()
    x = nc.dram_tensor("x", [M, DIM], F32, kind="ExternalInput")
    xyzT = nc.dram_tensor("xyzT", [3, M], F32R, kind="ExternalInput")
    wqk = nc.dram_tensor("wqk", [DIM, 256], F32R, kind="ExternalInput")
    wv = nc.dram_tensor("wv", [DIM, 128], F32R, kind="ExternalInput")
    wsp = nc.dram_tensor("wsp", [3, DH], F32R, kind="ExternalInput")
    bqk = nc.dram_tensor("bqk", [128, 2], F32, kind="ExternalInput")
    bv = nc.dram_tensor("bv", [128, 1], F32, kind="ExternalInput")
    idm = nc.dram_tensor("idm", [128, 128], F32R, kind="ExternalInput")
    otn = nc.dram_tensor("otn", [128, M], F32, kind="ExternalOutput")

    with tile.TileContext(nc) as tc:
      for _rep in range(repeat):
        with tc.tile_pool(name="consts", bufs=1) as consts, \
             tc.tile_pool(name="xall", bufs=1) as xall, \
             tc.tile_pool(name="stats", bufs=1) as stats, \
             tc.tile_pool(name="zp", bufs=3) as zp, \
             tc.tile_pool(name="ztp", bufs=1) as ztp, \
             tc.tile_pool(name="qkvsb", bufs=1) as qkvsb, \
             tc.tile_pool(name="esb", bufs=3) as esb, \
             tc.tile_pool(name="otp", bufs=1) as otp, \
             tc.tile_pool(name="small", bufs=4) as small:

            wqk_sb = consts.tile([128, KT, 256], F32R)
            wv_sb = consts.tile([128, KT, 128], F32R)
            wsp_sb = consts.tile([3, DH], F32R)
            xyzT_sb = consts.tile([3, M], F32R)
            bqk_sb = consts.tile([128, 2], F32)
            bv_sb = consts.tile([128, 1], F32)
            id_sb = consts.tile([128, 128], F32R)
            eps_sb = consts.tile([128, 1], F32)
            nc.sync.dma_start(out=wqk_sb, in_=wqk.rearrange("(k p) n -> p k n", p=128))
            nc.sync.dma_start(out=wv_sb, in_=wv.rearrange("(k p) n -> p k n", p=128))
            nc.sync.dma_start(out=wsp_sb, in_=wsp[:, :])
            nc.sync.dma_start(out=xyzT_sb, in_=xyzT[:, :])
            nc.sync.dma_start(out=bqk_sb, in_=bqk[:, :])
            nc.sync.dma_start(out=bv_sb, in_=bv[:, :])
            nc.sync.dma_start(out=id_sb, in_=idm[:, :])
            nc.vector.memset(eps_sb, EPS)

            x_sb = xall.tile([128, MT, DIM], F32)
            nc.sync.dma_start(out=x_sb, in_=x.rearrange("(t p) d -> p t d", p=128))

            # --- layernorm statistics for all 16 m-tiles ---
            st = stats.tile([128, MT, 6], F32)
            mv = stats.tile([128, MT, 2], F32)
            for t in range(MT):
                nc.vector.bn_stats(out=st[:, t, :], in_=x_sb[:, t, :])
                nc.vector.bn_aggr(out=mv[:, t, :], in_=st[:, t, :])
            rstd = stats.tile([128, MT], F32)
            nmr = stats.tile([128, MT], F32)
            nc.scalar.activation(rstd, mv[:, :, 1], SQRT, bias=eps_sb[:, 0:1])
            nc.vector.reciprocal(rstd, rstd)
            nc.vector.tensor_tensor(out=nmr, in0=mv[:, :, 0], in1=rstd, op=MULT)
            nc.vector.tensor_scalar(nmr, nmr, -1.0, None, op0=MULT)

            zt_sb = ztp.tile([128, KT, M], F32R)
            sb2 = qkvsb.tile([128, M], F32)
            qT = qkvsb.tile([128, M], F32R)
            kT = qkvsb.tile([128, M], F32R)
            vT = qkvsb.tile([128, M], F32R)
            vp_sb = qkvsb.tile([128, MT, 130], F32R)
            ones16 = qkvsb.tile([128, MT], F32)
            nc.vector.memset(ones16, 1.0)
            nc.vector.tensor_copy(vp_sb[:, :, 64], ones16[:, :])
            nc.vector.tensor_copy(vp_sb[:, :, 129], ones16[:, :])

            with tc.tile_pool(name="tps", bufs=3, space="PSUM") as tps, \
                 tc.tile_pool(name="qkps", bufs=3, space="PSUM") as qkps:
                # --- standardize (gpsimd) + transpose z tiles (PE) ---
                for t in range(MT):
                    z = zp.tile([128, DIM], F32)
                    nc.gpsimd.tensor_scalar(z, x_sb[:, t, :], rstd[:, t:t + 1],
                                            nmr[:, t:t + 1], op0=MULT, op1=ADD)
                    for k in range(KT):
                        pt = tps.tile([128, 128], F32, tag="tp")
                        nc.tensor.transpose(pt[:, :], z[:, 128 * k:128 * (k + 1)], id_sb[:, :].bitcast(F32))
                        nc.vector.tensor_copy(zt_sb[:, k, 128 * t:128 * (t + 1)], pt[:, :])

                # --- q,k,v projections (transposed layout [d, m]) ---
                for c in range(NCH):
                    cs = slice(512 * c, 512 * (c + 1))
                    pq = qkps.tile([128, 512], F32, tag="qk")
                    for k in range(KT):
                        nc.tensor.matmul(pq[:, :], wqk_sb[:, k, 0:128], zt_sb[:, k, cs],
                                         start=(k == 0), stop=(k == KT - 1))
                    nc.vector.tensor_scalar(qT[:, cs], pq[:, :], bqk_sb[:, 0:1], None, op0=ADD)
                    pk = qkps.tile([128, 512], F32, tag="qk")
                    for k in range(KT):
                        nc.tensor.matmul(pk[:, :], wqk_sb[:, k, 128:256], zt_sb[:, k, cs],
                                         start=(k == 0), stop=(k == KT - 1))
                    nc.vector.tensor_scalar(kT[:, cs], pk[:, :], bqk_sb[:, 1:2], None, op0=ADD)
                    # s_b^T = (xyz @ w_sp)^T, duplicated into both head halves
                    psb = qkps.tile([128, 512], F32, tag="qk")
                    nc.tensor.matmul(psb[0:64, :], wsp_sb[:, :], xyzT_sb[:, cs],
                                     start=True, stop=True)
                    nc.vector.tensor_copy(sb2[0:64, cs], psb[0:64, :])
                    nc.vector.tensor_copy(sb2[64:128, cs], psb[0:64, :])
                    pv = qkps.tile([128, 512], F32, tag="qk")
                    for k in range(KT):
                        nc.tensor.matmul(pv[:, :], wv_sb[:, k, :], zt_sb[:, k, cs],
                                         start=(k == 0), stop=(k == KT - 1))
                    # v' = v + bias_v + s_b  (one fused DVE op)
                    nc.vector.scalar_tensor_tensor(vT[:, cs], pv[:, :], bv_sb[:, 0:1],
                                                   sb2[:, cs], op0=ADD, op1=ADD)

                # --- transpose v' back to natural [j, d] layout with ones cols ---
                for t in range(MT):
                    pt = tps.tile([128, 128], F32R, tag="tp")
                    nc.tensor.transpose(pt[:, :], vT[:, 128 * t:128 * (t + 1)],
                                        id_sb[:, :])
                    nc.vector.tensor_copy(vp_sb[:, t, 0:64], pt[0:128, 0:64])
                    nc.vector.tensor_copy(vp_sb[:, t, 65:129], pt[0:128, 64:128])

            # --- attention: S^T tiles -> exp -> accumulate (P@v')^T ---
            # slots q=0..31 per i-chunk: (j, head) = (q//2, q%2); exp runs over
            # groups of 3 slots (1536 cols) to amortize the ACT per-op overhead.
            ot_sb = otp.tile([128, M], F32)
            with tc.tile_pool(name="sab", bufs=2, space="PSUM") as sabp, \
                 tc.tile_pool(name="ops", bufs=1, space="PSUM") as ops:
                for c in range(NCH):
                    cs = slice(512 * c, 512 * (c + 1))
                    oa = ops.tile([128, 512], F32, tag="oa")
                    ob = ops.tile([128, 512], F32, tag="ob")
                    for g in range(11):
                        slots = list(range(3 * g, min(3 * g + 3, 2 * MT)))
                        sab = sabp.tile([128, 1536], F32, tag="sab")
                        for idx, q in enumerate(slots):
                            j, h = divmod(q, 2)
                            js = slice(128 * j, 128 * (j + 1))
                            nc.tensor.matmul(sab[:, 512 * idx:512 * (idx + 1)],
                                             kT[64 * h:64 * (h + 1), js],
                                             qT[64 * h:64 * (h + 1), cs],
                                             start=True, stop=True)
                        w = 512 * len(slots)
                        e = esb.tile([128, 1536], F32R, tag="e")
                        nc.scalar.activation(e[:, 0:w], sab[:, 0:w], EXP, scale=SCALE)
                        for idx, q in enumerate(slots):
                            j, h = divmod(q, 2)
                            o = oa if h == 0 else ob
                            nc.tensor.matmul(o[0:65, :], vp_sb[:, j, 65 * h:65 * h + 65],
                                             e[:, 512 * idx:512 * (idx + 1)],
                                             start=(q < 2), stop=(q >= 2 * MT - 2),
                                             skip_group_check=True)
                    ra = small.tile([1, 512], F32, tag="r")
                    rb = small.tile([1, 512], F32, tag="r")
                    nc.vector.reciprocal(ra, oa[64:65, :])
                    nc.vector.reciprocal(rb, ob[64:65, :])
                    rab = small.tile([64, 512], F32, tag="rb")
                    rbb = small.tile([64, 512], F32, tag="rb")
                    nc.gpsimd.partition_broadcast(rab, ra[0:1, :])
                    nc.gpsimd.partition_broadcast(rbb, rb[0:1, :])
                    nc.vector.tensor_tensor(out=ot_sb[0:64, cs], in0=oa[0:64, :], in1=rab, op=MULT)
                    nc.vector.tensor_tensor(out=ot_sb[64:128, cs], in0=ob[0:64, :], in1=rbb, op=MULT)
                    nc.sync.dma_start(out=otn[:, cs], in_=ot_sb[:, cs])
    nc.compile()
    return nc


def _build_launch2(repeat: int = 1) -> bass.Bass:
    """Per core: 512 token rows.  y^T = gelu(w_out^T @ ot - wfold^T @ s_b^T
    + b_out) + x^T, all in [dim, i] layout."""
    nc = bacc.Bacc()
    ot = nc.dram_tensor("ot", [INNER, 512], F32R, kind="ExternalInput")
    wout = nc.dram_tensor("wout", [INNER, DIM], F32R, kind="ExternalInput")
    wfold = nc.dram_tensor("wfold", [DH, DIM], F32R, kind="ExternalInput")
    xyzTs = nc.dram_tensor("xyzTs", [3, 512], F32R, kind="ExternalInput")
    wsp = nc.dram_tensor("wsp", [3, DH], F32R, kind="ExternalInput")
    bout = nc.dram_tensor("bout", [128, KT], F32, kind="ExternalInput")
    xT = nc.dram_tensor("xT", [DIM, 512], F32, kind="ExternalInput")
    y = nc.dram_tensor("y", [DIM, 512], F32, kind="ExternalOutput")

    with tile.TileContext(nc) as tc:
      for _rep in range(repeat):
        with tc.tile_pool(name="sb", bufs=1) as sb, \
             tc.tile_pool(name="ps", bufs=2, space="PSUM") as ps, \
             tc.tile_pool(name="ps1", bufs=1, space="PSUM") as ps1:
            ot_sb = sb.tile([128, KT, 512], F32R)
            wout_sb = sb.tile([128, KT, DIM], F32R)
            wfold_sb = sb.tile([DH, DIM], F32R)
            xyzT_sb = sb.tile([3, 512], F32R)
            wsp_sb = sb.tile([3, DH], F32R)
            bout_sb = sb.tile([128, KT], F32)
            xT_sb = sb.tile([128, KT, 512], F32)
            y_sb = sb.tile([128, KT, 512], F32)
            nc.sync.dma_start(out=ot_sb, in_=ot.rearrange("(k p) n -> p k n", p=128))
            nc.sync.dma_start(out=wout_sb, in_=wout.rearrange("(k p) n -> p k n", p=128))
            nc.sync.dma_start(out=wfold_sb, in_=wfold[:, :])
            nc.sync.dma_start(out=xyzT_sb, in_=xyzTs[:, :])
            nc.sync.dma_start(out=wsp_sb, in_=wsp[:, :])
            nc.sync.dma_start(out=bout_sb, in_=bout[:, :])
            nc.sync.dma_start(out=xT_sb, in_=xT.rearrange("(k p) n -> p k n", p=128))

            sbt_ps = ps1.tile([128, 512], F32)
            nc.tensor.matmul(sbt_ps[0:DH, :], wsp_sb[:, :], xyzT_sb[:, :],
                             start=True, stop=True)
            sbt = sb.tile([DH, 512], F32R)
            nc.vector.tensor_copy(sbt, sbt_ps[0:DH, :])

            for d in range(KT):
                ds = slice(128 * d, 128 * (d + 1))
                py = ps.tile([128, 512], F32, tag="y")
                for k in range(KT):
                    nc.tensor.matmul(py[:, :], wout_sb[:, k, ds], ot_sb[:, k, :],
                                     start=(k == 0), stop=False, skip_group_check=True)
                nc.tensor.matmul(py[:, :], wfold_sb[:, ds], sbt[:, :],
                                 start=False, stop=True, skip_group_check=True)
                g = sb.tile([128, 512], F32)
                nc.scalar.activation(g, py[:, :], GELU, bias=bout_sb[:, d:d + 1])
                nc.vector.tensor_tensor(out=y_sb[:, d, :], in0=g, in1=xT_sb[:, d, :], op=ADD)
            nc.sync.dma_start(out=y.rearrange("(k p) n -> p k n", p=128), in_=y_sb)
    nc.compile()
    return nc


_NC1 = None
_NC2 = None


def prepare_inputs1(xyzs, features, ln_gamma, ln_beta, w_qkv, w_sp, w_out, b_out):
    wp = ln_gamma[:, None] * w_qkv                      # [512, 1536]
    bias_full = ln_beta @ w_qkv                         # [1536]
    idm = np.eye(128, dtype=np.float32)
    xf = features.reshape(B, M, DIM)
    xyzf = xyzs.reshape(B, M, 3)
    in1 = []
    for c in range(NCORES):
        b, p = divmod(c, 4)
        h0 = 2 * p
        qcols = [wp[:, 64 * h:64 * (h + 1)] for h in (h0, h0 + 1)]
        kcols = [wp[:, INNER + 64 * h:INNER + 64 * (h + 1)] for h in (h0, h0 + 1)]
        vcols = [wp[:, 2 * INNER + 64 * h:2 * INNER + 64 * (h + 1)] for h in (h0, h0 + 1)]
        bq = np.concatenate([bias_full[64 * h:64 * (h + 1)] for h in (h0, h0 + 1)])
        bk = np.concatenate([bias_full[INNER + 64 * h:INNER + 64 * (h + 1)]
                             for h in (h0, h0 + 1)])
        bvv = np.concatenate([bias_full[2 * INNER + 64 * h:2 * INNER + 64 * (h + 1)]
                              for h in (h0, h0 + 1)])
        in1.append({
            "x": np.ascontiguousarray(xf[b]),
            "xyzT": np.ascontiguousarray(xyzf[b].T),
            "wqk": np.ascontiguousarray(np.concatenate(qcols + kcols, axis=1)),
            "wv": np.ascontiguousarray(np.concatenate(vcols, axis=1)),
            "wsp": np.ascontiguousarray(w_sp),
            "bqk": np.ascontiguousarray(np.stack([bq, bk], axis=1)),
            "bv": np.ascontiguousarray(bvv[:, None]),
            "idm": idm,
        })
    return in1


def assemble_mid(results1):
    otf = np.empty((B, INNER, M), dtype=np.float32)
    for c in range(NCORES):
        b, p = divmod(c, 4)
        otf[b, 128 * p:128 * (p + 1)] = results1[c]["otn"]
    return otf


def prepare_inputs2(xyzs, features, ln_gamma, ln_beta, w_qkv, w_sp, w_out, b_out,
                    otf):
    xf = features.reshape(B, M, DIM)
    xyzf = xyzs.reshape(B, M, 3)
    wfold = -w_out.reshape(H, DH, DIM).sum(axis=0)      # [64, 512]
    in2 = []
    for c in range(NCORES):
        b, p = divmod(c, 4)
        isl = slice(512 * p, 512 * (p + 1))
        in2.append({
            "ot": np.ascontiguousarray(otf[b][:, isl]),
            "wout": np.ascontiguousarray(w_out),
            "wfold": np.ascontiguousarray(wfold),
            "xyzTs": np.ascontiguousarray(xyzf[b].T[:, isl]),
            "wsp": np.ascontiguousarray(w_sp),
            "bout": np.ascontiguousarray(b_out.reshape(KT, 128).T),
            "xT": np.ascontiguousarray(xf[b].T[:, isl]),
        })
    return in2


def assemble_out(results2):
    out = np.empty((B, M, DIM), dtype=np.float32)
    for c in range(NCORES):
        b, p = divmod(c, 4)
        out[b, 512 * p:512 * (p + 1)] = results2[c]["y"].T
    return out.reshape(B, L, N, DIM)


_NC1 = None
_NC2 = None


def get_nc1():
    global _NC1
    if _NC1 is None:
        _NC1 = _build_launch1()
    return _NC1


def get_nc2():
    global _NC2
    if _NC2 is None:
        _NC2 = _build_launch2()
    return _NC2


def kernel(xyzs, features, ln_gamma, ln_beta, w_qkv, w_sp, w_out, b_out):
    args = [np.asarray(a, dtype=np.float32) for a in
            (xyzs, features, ln_gamma, ln_beta, w_qkv, w_sp, w_out, b_out)]
    in1 = prepare_inputs1(*args)
    r1 = run_bass_kernel_spmd(get_nc1(), in1, core_ids=list(range(NCORES)))
    otf = assemble_mid(r1.results)
    in2 = prepare_inputs2(*args, otf)
    r2 = run_bass_kernel_spmd(get_nc2(), in2, core_ids=list(range(NCORES)))
    return assemble_out(r2.results)
